# revision 27
# baseline (speedup 1.0000x reference)
# Trainium2 Bass kernel for nn_CoefficientLayer (per-species MLP dispatch,
# ANI-style). Strategy: MoE routing done on host (sort atoms by species, pad
# each species group so all 8 cores get an identical tile schedule of 512/384/
# 256-atom tiles), device runs a dense 4-layer MLP per tile with the tile's
# species' weights selected statically at build time, emitted as a 4-deep
# software pipeline ([L1(t+3), L2(t+2), L3(t+1), L4(t)] per round) so the PE
# never stalls on the activation chain.
#
# Device math (per tile, feature-major, fp32r matmuls, fp32 psum), e = exp(1):
#   stored Hb_k = e*(elu(y_k)+1),  y_k = x_k/alpha,  using
#     e*(elu(y)+1) = max(e*(y+1), min(exp(y+1), e))
#   psum_k = e*(y_k+1) via lhsT chunks of plain W_k plus an augmented
#   bias row e*(beta_k+1) (own ones-row chunk for L1/L2; free zero-pad row of
#   the k1 chunk for L3).  Per hidden layer (both m-chunks consolidated):
#     ACT:  E = Exp(psum * (1/e))          [128, 2, n]
#     DVE:  Hb = (E min e) max psum        (scalar_tensor_tensor)
#   Zero-padded weight columns make pad rows of Hb exactly 1.0, which both
#   feeds L3's augmented row and is killed by zero rows in the next lhsT.
#   L4: psum4 = (alpha/e)*W4^T Hb3, coef = psum4 + alpha*beta4, then the
#   shifter folds into one ACT: out = s1*psum4 + (s0 + s1*alpha*beta4).
import numpy as np
from contextlib import ExitStack

import concourse.bass as bass
import concourse.tile as tile
from concourse import bacc, mybir
from concourse.bass_utils import run_bass_kernel_spmd

ALPHA = 0.1
E1 = float(np.exp(1.0))
P = 128
NCORES = 8
QUANTUM = 128
DIN = 384
DIMS = [384, 256, 192, 160]

F32 = mybir.dt.float32
F32R = mybir.dt.float32r
AF = mybir.ActivationFunctionType
ALU = mybir.AluOpType

# layer -> (n_weight_chunks_per_m (incl aug), n_m_chunks, true_K, true_M)
CHUNKS = {1: (4, 2, 384, 256), 2: (3, 2, 256, 192), 3: (2, 2, 192, 160),
          4: (2, 1, 160, 1)}
WCOLS_PER_S = (8 + 6 + 4) * P + 2
BCOLS_PER_S = 1  # shifter bias


def _wcol(s, layer, m, k):
    off = s * WCOLS_PER_S
    for l in (1, 2, 3):
        nk, nm = CHUNKS[l][0], CHUNKS[l][1]
        if l == layer:
            return off + (m * nk + k) * P
        off += nk * nm * P
    assert layer == 4 and m == 0
    return off + k


def _fold_host(inputs):
    """Pack weight image [128, 4*WCOLS_PER_S] and shifter consts."""
    al = ALPHA
    wimg = np.zeros((P, 4 * WCOLS_PER_S), dtype=np.float32)
    bimg = np.zeros((P, 4 * BCOLS_PER_S), dtype=np.float32)
    shifter_scale = []
    for s in range(4):
        W = [np.asarray(inputs[f"W{i}"][s], np.float32) for i in (1, 2, 3, 4)]
        b = [np.asarray(inputs[f"b{i}"][s], np.float32) for i in (1, 2, 3, 4)]
        Wt = [(E1 / al) * W[0], W[1], W[2], (al / E1) * W[3]]
        aug = [E1 * (b[0] / al + 1.0),
               E1 * (b[1] / al - W[1].sum(axis=0) + 1.0),
               E1 * (b[2] / al - W[2].sum(axis=0) + 1.0)]
        beta4 = b[3] - al * W[3].sum(axis=0)          # al*beta4 = b4 - al*colsum

        for layer in (1, 2, 3, 4):
            nk, nm, tk, tm = CHUNKS[layer]
            Wl = Wt[layer - 1]
            w = 1 if layer == 4 else P
            for m in range(nm):
                mlo, mhi = m * P, min((m + 1) * P, tm)
                for k in range(nk):
                    blk = np.zeros((P, w), np.float32)
                    is_aug = (layer in (1, 2)) and (k == nk - 1)
                    if is_aug:
                        blk[0, :mhi - mlo] = aug[layer - 1][mlo:mhi]
                    else:
                        rows = Wl[k * P:min((k + 1) * P, tk), mlo:mhi]
                        blk[:rows.shape[0], :rows.shape[1]] = rows
                        if layer == 3 and k == 1:
                            # aug row rides the zero-pad row 64 (Hb2m1 pad = 1)
                            blk[64, :mhi - mlo] = aug[2][mlo:mhi]
                    wimg[:, _wcol(s, layer, m, k):_wcol(s, layer, m, k) + w] = blk

        s1 = float(np.asarray(inputs["shift_b1"], np.float32)[s])
        s0 = float(np.asarray(inputs["shift_b0"], np.float32)[s])
        bimg[:, s] = s0 + s1 * float(beta4[0])
        shifter_scale.append(s1)
    return wimg, bimg, shifter_scale


def _host_prepare(inputs):
    species = np.asarray(inputs["species"]).ravel()
    aev = np.ascontiguousarray(np.asarray(inputs["aev"], np.float32).reshape(-1, DIN))
    order = np.argsort(species, kind="stable")
    counts = np.bincount(species, minlength=4)
    a = np.maximum(np.ceil(counts / (NCORES * QUANTUM)), 2).astype(int) * QUANTUM
    A_pc = int(a.sum())

    idx = np.full((NCORES, A_pc), -1, dtype=np.int64)
    off_sorted = 0
    off_core = 0
    for s in range(4):
        grp = order[off_sorted:off_sorted + counts[s]]
        for c in range(NCORES):
            lo = min(counts[s], c * a[s])
            hi = min(counts[s], (c + 1) * a[s])
            idx[c, off_core:off_core + (hi - lo)] = grp[lo:hi]
        off_sorted += counts[s]
        off_core += a[s]

    aev_t = np.zeros((NCORES, DIN, A_pc), dtype=np.float32)
    for c in range(NCORES):
        valid = idx[c] >= 0
        aev_t[c][:, valid] = aev[idx[c][valid]].T

    sched = []
    off = 0
    for s in range(4):
        rem = int(a[s])
        col = off
        while rem > 0:
            # keep every tile >= 256 atoms (fp32r full-rate needs N >= 256)
            if rem in (640, 384):
                n = rem - 256
            elif rem >= 512:
                n = 512
            else:
                n = rem
            assert n >= 256 or rem == n, (rem, n)
            sched.append((s, col, n))
            col += n
            rem -= n
        off += int(a[s])
    return aev_t, idx, sched, A_pc


def _build_program(sched, A_pc, shifter_scale):
    nc = bacc.Bacc("TRN2", target_bir_lowering=False, debug=False)
    aev_d = nc.dram_tensor("aev_t", [DIN, A_pc], F32R, kind="ExternalInput").ap()
    w_d = nc.dram_tensor("wimg", [P, 4 * WCOLS_PER_S], F32R, kind="ExternalInput").ap()
    b_d = nc.dram_tensor("bimg", [P, 4 * BCOLS_PER_S], F32, kind="ExternalInput").ap()
    out_d = nc.dram_tensor("out", [1, A_pc], F32, kind="ExternalOutput").ap()

    with tile.TileContext(nc) as tc, ExitStack() as ctx:
        wpool = ctx.enter_context(tc.tile_pool(name="w", bufs=1))
        xpool = ctx.enter_context(tc.tile_pool(name="x", bufs=4))
        hpool = ctx.enter_context(tc.tile_pool(name="h", bufs=3))
        epool = ctx.enter_context(tc.tile_pool(name="e", bufs=4))

        pspool = ctx.enter_context(tc.tile_pool(name="ps", bufs=1, space="PSUM"))
        ps4pool = ctx.enter_context(tc.tile_pool(name="ps4", bufs=2, space="PSUM"))

        # per-(species, layer) weight tiles; species 0's L1 chunks load
        # first (the prologue is DMA-bandwidth-bound), everything else is
        # deferred into the pipeline rounds
        lay_cols = {1: 8 * P, 2: 6 * P, 3: 4 * P, 4: 2}
        lay_off = {1: 0, 2: 8 * P, 3: 14 * P, 4: 18 * P}
        wtiles = {}

        def load_weights(sp, layers=(1, 2, 3, 4)):
            for ly in layers:
                wt = wpool.tile([P, lay_cols[ly]], F32R, tag=f"w{sp}L{ly}")
                c0 = sp * WCOLS_PER_S + lay_off[ly]
                nc.sync.dma_start(wt[:], w_d[:, c0:c0 + lay_cols[ly]])
                wtiles[(sp, ly)] = wt

        load_weights(0, layers=(1,))
        bsb = wpool.tile([P, 4 * BCOLS_PER_S], F32, tag="bimg")
        nc.sync.dma_start(bsb[:], b_d[:])
        ystage = wpool.tile([1, A_pc], F32, tag="ystage")
        ones_f = wpool.tile([P, 512], F32, tag="ones_f")
        nc.vector.memset(ones_f[:], 1.0)
        ones = wpool.tile([P, 512], F32R, tag="ones")
        nc.vector.tensor_copy(ones[:], ones_f[:])

        def wsl(s, layer, m, k, width=P):
            c0 = _wcol(s, layer, m, k) - s * WCOLS_PER_S - lay_off[layer]
            return wtiles[(s, layer)][:, c0:c0 + width]

        # 4-deep software pipeline: in one emission round the PE stream is
        # [L1(t+3), L2(t+2), L3(t+1), L4(t)], so each layer's exp->stt chain
        # elapses while the PE runs the other tiles' matmuls (no PE stalls).
        T = len(sched)
        hid = {}   # (tile, layer) -> hidden tile handle
        xloads = {}  # tile -> list of SBUF x-chunk APs

        def stage_load(t):
            s, col, n = sched[t]
            xt = xpool.tile([P, 3, 512], F32R, tag="x")
            src = aev_d.rearrange("(k p) a -> p k a", k=3)
            nc.sync.dma_start(xt[:, :, :n], src[:, :, col:col + n])
            xloads[t] = [xt[:, k, :n] for k in range(3)]

        def stage_hidden(t, layer):
            """Matmuls + exp + stt for `layer` (1..3) of tile t."""
            s, col, n = sched[t]
            if layer == 1:
                hs = xloads.pop(t)
            else:
                prev = hid.pop((t, layer - 1))
                hs = [prev[:, 0, :n], prev[:, 1, :n]]
            nk, nm, tk, tm = CHUNKS[layer]
            n_real = nk - 1 if layer in (1, 2) else nk
            ps = pspool.tile([P, 2, 512], F32, tag=f"ps{layer}")
            # interleave m0/m1 k-chunks: one LDW/MM chain per stage, both
            # psum groups stream after a single stage-start wait
            for k in range(n_real):
                for m in range(nm):
                    nc.tensor.matmul(ps[:, m, :n], wsl(s, layer, m, k), hs[k],
                                     start=(k == 0),
                                     stop=(k == n_real - 1 and layer == 3))
            if layer in (1, 2):  # augmented ones-row bias chunks
                for m in range(nm):
                    nc.tensor.matmul(ps[:, m, :n], wsl(s, layer, m, n_real),
                                     ones[:, :n], start=False, stop=True)
            et = epool.tile([P, 2, 512], F32, tag="e")
            nc.scalar.activation(et[:, :, :n], ps[:, :, :n], AF.Exp,
                                 bias=0.0, scale=1.0 / E1)
            ht = hpool.tile([P, 2, 512], F32R, tag=f"h{layer}")
            nc.vector.scalar_tensor_tensor(
                ht[:, :, :n], et[:, :, :n], E1, ps[:, :, :n],
                ALU.min, ALU.max)
            hid[(t, layer)] = ht

        species_last = {}
        species_range = {}
        for i, (sp, c0, nn_) in enumerate(sched):
            species_last[sp] = i
            lo, hi = species_range.get(sp, (c0, c0))
            species_range[sp] = (min(lo, c0), max(hi, c0 + nn_))

        def stage_out(t):
            """L4 matmuls + shifter + output flush for tile t."""
            s, col, n = sched[t]
            h3 = hid.pop((t, 3))
            ps4 = ps4pool.tile([1, 512], F32, tag="ps4")
            nc.tensor.matmul(ps4[:, :n], wsl(s, 4, 0, 0, width=1),
                             h3[:, 0, :n], start=True, stop=False)
            nc.tensor.matmul(ps4[:, :n], wsl(s, 4, 0, 1, width=1),
                             h3[:, 1, :n], start=False, stop=True)
            nc.scalar.activation(ystage[:, col:col + n], ps4[:, :n],
                                 AF.Identity,
                                 bias=bsb[0:1, s:s + 1], scale=shifter_scale[s])
            if species_last[s] == t:  # flush this species' outputs (overlaps)
                lo, hi = species_range[s]
                nc.sync.dma_start(out_d[:, lo:hi], ystage[:, lo:hi])

        # species s first needed at the L1 stage of its first tile; emit its
        # weight load ~3 rounds earlier
        first_tile = {}
        for i, (sp, _, _) in enumerate(sched):
            first_tile.setdefault(sp, i)
        wload_round = {max(-4, first_tile[sp] - 3 - 3): sp
                       for sp in sorted(first_tile) if sp != 0}

        for t in range(-5, T):
            if t == -3:
                load_weights(0, layers=(2, 3, 4))
            if t in wload_round:
                load_weights(wload_round[t])
            if 0 <= t + 5 < T:
                stage_load(t + 5)
            if 0 <= t + 3 < T:
                stage_hidden(t + 3, 1)
            if 0 <= t + 2 < T:
                stage_hidden(t + 2, 2)
            if 0 <= t + 1 < T:
                stage_hidden(t + 1, 3)
            if 0 <= t < T:
                stage_out(t)

    nc.compile()
    return nc


def kernel(**inputs):
    species = np.asarray(inputs["species"])
    out_dtype = np.asarray(inputs["aev"]).dtype
    aev_t, idx, sched, A_pc = _host_prepare(inputs)
    wimg, bimg, shifter_scale = _fold_host(inputs)
    nc = _build_program(sched, A_pc, shifter_scale)

    in_maps = [{"aev_t": np.ascontiguousarray(aev_t[c]), "wimg": wimg, "bimg": bimg}
               for c in range(NCORES)]
    res = run_bass_kernel_spmd(nc, in_maps, core_ids=list(range(NCORES)))

    out = np.zeros(species.size, dtype=np.float32)
    for c in range(NCORES):
        valid = idx[c] >= 0
        out[idx[c][valid]] = res.results[c]["out"][0][valid]
    return out.reshape(species.shape).astype(out_dtype, copy=False)
